# revision 19
# baseline (speedup 1.0000x reference)
"""ConvCrossAttention Trainium2 kernel — self-contained.

Problem (B=4, C_in=C_out=256, H=W=64, N=4096):
  q = conv1x1(x1, Wq, bq); k = conv1x1(x2, Wk, bk); v = conv1x1(x2, Wv, bv)
  out = softmax(q^T k / sqrt(C)) @ v^T, back in conv layout [B, C, H, W].

Sharding: data-parallel over (batch, query-half) -> 8 NeuronCores.
Core c handles batch c//2, query rows (c%2)*2048 : (c%2+1)*2048, with the
full 4096-key context for that batch. No collectives.

Weight fusion (host side): softmax over keys is invariant to per-query
additive constants, so
  S_nm =(softmax) (A^T x1_n + c) . x2_m   with A = Wq^T Wk, c = Wk^T bq.
The K projection disappears entirely (raw x2 is the key matrix) and the
q projection uses the fused A instead of Wq.

Per-core program (everything SBUF-resident):
  Warm-up: fp32 dummy matmuls during the input-DMA head keep the PE busy
  so the HAM clock gate reaches 8/8 (2.4 GHz) before real work.
  DMA: three descriptor rings (Sync / Activation / Pool), each leading
  with its critical tensor: [aT, x2 q2, x1 rest], [wv, x1c0, x2 q4, bv],
  [x2 q1, cq, x2 q3]; inputs land in consumption order at the ~358 GB/s
  aggregate limit.
  Prologue: all 16 V-pair projections (fp8 out) + the 4 q projections
  stream behind the DMA.
  Main loop: 4 query chunks, flash-style, in 2-key-tile pairs: S^T =
  x2^T q (PE, f32r, two banks of one wide PSUM slot), P = exp(S/16)
  (ACT, ONE 1024-wide activation per pair, fp8e4 out; |scores|/16 < ~5
  so no max-subtraction, p_max << 448), PV via fp8 DoubleRow matmuls
  (0.5 cyc/row), and the softmax denominator accumulated ON THE PE by a
  third DoubleRow matmul with an fp8 ones stationary (no elementwise
  P-sum tree anywhere). The previous chunk's last PV pairs + tail (fold
  matmuls closing the accumulation, reciprocal, Pool-engine partition
  broadcast, normalize, out-DMA) are woven into pairs 1..3 of the next
  chunk's S stream so the in-order PE queue never stalls.

S matmuls stay float32r (1 cyc/row); dropping scores to fp8 would cost
~1.7e-2 relative error (measured off-line) against the 2e-2 budget, while
fp8 P/V costs only ~1e-2. Softmax denominators use reciprocal_approx_fast
(~18-bit); inputs are sums of positive exps so its undefined edge cases
(0/denorm/inf) cannot occur.
"""

import sys

if "/opt/trn_rl_repo" not in sys.path:
    sys.path.insert(0, "/opt/trn_rl_repo")

from contextlib import ExitStack

import numpy as np

import concourse.bass as bass  # noqa: F401
import concourse.mybir as mybir
import concourse.tile as tile
from concourse import bacc
from concourse.bass_utils import run_bass_kernel_spmd

F32 = mybir.dt.float32
F32R = mybir.dt.float32r
F8 = mybir.dt.float8e4
DR = mybir.MatmulPerfMode.DoubleRow

B, C, H, W = 4, 256, 64, 64
N = H * W  # 4096
NQ = 2048  # queries per core (half a batch)
NK = 4096  # full key context
CHUNK = 512
NQ_CHUNKS = NQ // CHUNK
NK_TILES = NK // 128  # 32
NPAIRS = NK_TILES // 2  # 16 fp8 DoubleRow PV pairs
SCALE = 1.0 / 16.0  # C ** -0.5
WARMUP_MMS = 3  # fp32 dummy matmuls (~1.7us each) bridging the DMA head
PVTRAIL = 4  # PV pairs trail S pairs by this much


def build_nc():
    MM = F32R
    nc = bacc.Bacc(None, debug=False)

    x1 = nc.dram_tensor("x1c", [C, NQ], MM, kind="ExternalInput")
    x2 = nc.dram_tensor("x2c", [C, NK], MM, kind="ExternalInput")
    at = nc.dram_tensor("aT", [C, C], MM, kind="ExternalInput")  # A = Wq^T Wk
    wv = nc.dram_tensor("wvT", [C, C], MM, kind="ExternalInput")
    cq = nc.dram_tensor("cq", [C, 1], F32, kind="ExternalInput")  # Wk^T bq
    bv = nc.dram_tensor("bv", [C, 1], F32, kind="ExternalInput")
    out = nc.dram_tensor("out", [C, NQ], F32, kind="ExternalOutput")

    def split_h(ap):  # DRAM [256, w] -> [128, 2, w] (partition-first)
        return ap.rearrange("(h p) w -> p h w", p=128)

    with tile.TileContext(nc) as tc, ExitStack() as ctx:
        big = ctx.enter_context(tc.tile_pool(name="big", bufs=1))
        small = ctx.enter_context(tc.tile_pool(name="small", bufs=1))
        ppool = ctx.enter_context(tc.tile_pool(name="p", bufs=6))
        opool = ctx.enter_context(tc.tile_pool(name="o", bufs=2))
        dpool = ctx.enter_context(tc.tile_pool(name="d", bufs=2))
        # PSUM: 2 wide S slots (2 banks each) + 3 acc + 1 den = 8 banks
        spsum = ctx.enter_context(tc.tile_pool(name="spsum", bufs=2, space="PSUM"))
        apsum = ctx.enter_context(tc.tile_pool(name="apsum", bufs=3, space="PSUM"))
        dpsum = ctx.enter_context(tc.tile_pool(name="dpsum", bufs=1, space="PSUM"))

        # --- SBUF residents ---
        a_sb = small.tile([128, 2, C], MM, tag="a")
        wv_sb = small.tile([128, 2, C], MM, tag="wv")
        cq_sb = small.tile([128, 2, 1], F32, tag="cq")
        x1_sb = big.tile([128, 2, NQ], MM, tag="x1")
        x2_sb = big.tile([128, 2, NK], MM, tag="x2")
        q_sb = big.tile([128, 2, NQ], MM, tag="q")
        v_sb = big.tile([128, NPAIRS, 2, C], F8, tag="v")
        wu = small.tile([128, 512], F32, tag="wu")
        ones_pair_f32 = small.tile([128, 2, 16], F32, tag="ones_pair_f32")
        # 16 identical weight columns: DoubleRow LDWEIGHTS needs 16B-aligned
        # interleave steps, so a [128,2,1] ones vector is invalid ISA
        ones_pair = small.tile([128, 2, 16], F8, tag="ones_pair")
        bv_row = small.tile([1, 2, 128], MM, tag="bv_row")

        # --- DMA triggers. Three rings (Sync / Activation HWDGE, Pool
        # SWDGE) drain in FIFO order each, so every ring leads with its
        # critical tensor. x2 goes in 1024-col quarters (4KB descriptor
        # runs) split across all three rings. ---
        nc.gpsimd.memset(wu[:], 0.0)
        nc.gpsimd.dma_start(out=x2_sb[:, :, 0:1024], in_=split_h(x2[:, 0:1024]))
        nc.gpsimd.dma_start(out=cq_sb[:], in_=split_h(cq[:, :]))
        nc.gpsimd.memset(ones_pair_f32[:], 1.0)
        nc.vector.tensor_copy(ones_pair[:], ones_pair_f32[:])
        nc.gpsimd.dma_start(out=x2_sb[:, :, 2048:3072], in_=split_h(x2[:, 2048:3072]))

        nc.sync.dma_start(out=a_sb[:], in_=split_h(at[:, :]))
        nc.sync.dma_start(out=x2_sb[:, :, 1024:2048], in_=split_h(x2[:, 1024:2048]))
        nc.sync.dma_start(out=x1_sb[:, :, CHUNK:NQ], in_=split_h(x1[:, CHUNK:NQ]))

        nc.scalar.dma_start(out=wv_sb[:], in_=split_h(wv[:, :]))
        nc.scalar.dma_start(out=x1_sb[:, :, 0:CHUNK], in_=split_h(x1[:, 0:CHUNK]))
        nc.scalar.dma_start(out=x2_sb[:, :, 3072:4096], in_=split_h(x2[:, 3072:4096]))
        nc.scalar.dma_start(
            out=bv_row[:], in_=bv[:, :].rearrange("(h p) o -> o h p", p=128).bitcast(F32R)
        )

        # --- HAM warm-up: fp32 dummy matmuls (4 cyc/row, ~1.7us each)
        # with no input dependency bridge the DMA head so the PE reaches
        # the 8/8 clock before, and stays busy until, real work starts ---
        for _ in range(WARMUP_MMS):
            wup = dpsum.tile([128, 512], F32, tag="db", name="wup")
            nc.tensor.matmul(wup[:], wu[:, 0:128], wu[:], start=True, stop=True)

        # --- projection helpers (prologue; share the wide S slots one
        # bank at a time) ---
        def qproj(c0):
            cs = slice(c0 * CHUNK, (c0 + 1) * CHUNK)
            for ct in range(2):
                qp = spsum.tile([128, 2, CHUNK], F32, tag="s", name="qp")
                cts = slice(ct * 128, (ct + 1) * 128)
                nc.tensor.matmul(qp[:, 0, :], a_sb[:, 0, cts], x1_sb[:, 0, cs], start=True, stop=False)
                nc.tensor.matmul(qp[:, 0, :], a_sb[:, 1, cts], x1_sb[:, 1, cs], start=False, stop=True)
                nc.vector.tensor_scalar_add(q_sb[:, ct, cs], qp[:, 0, :], cq_sb[:, ct, :])

        def vproj_pair(u):
            # two 128-key tiles share one PSUM bank -> one wide fp8 copy out
            vp = spsum.tile([128, 2, C], F32, tag="s", name="vp")
            for s2 in range(2):
                t = 2 * u + s2
                ts_ = slice(t * 128, (t + 1) * 128)
                nc.tensor.matmul(
                    vp[:, s2, :], x2_sb[:, 0, ts_], wv_sb[:, 0, :],
                    start=(s2 == 0), stop=False,
                )
                nc.tensor.matmul(
                    vp[:, s2, :], x2_sb[:, 1, ts_], wv_sb[:, 1, :],
                    start=False, stop=(s2 == 1),
                )
            if u % 2 == 0:
                nc.scalar.copy(v_sb[:, u, :, :], vp[:])
            else:
                nc.vector.tensor_copy(v_sb[:, u, :, :], vp[:])

        # --- attention chunk state ---
        class ChunkState:
            def __init__(self, c0):
                self.c0 = c0
                self.cs = slice(c0 * CHUNK, (c0 + 1) * CHUNK)
                self.acc0 = apsum.tile([128, CHUNK], F32, tag="acc", name="acc0")
                self.acc1 = apsum.tile([128, CHUNK], F32, tag="acc", name="acc1")
                # softmax denominator accumulates on the PE: one DoubleRow
                # matmul per pair with a [128,2,16] fp8 ones stationary sums
                # P over keys into rows 0..15 of this PSUM bank (no
                # elementwise P-sum tree on DVE/Pool at all)
                self.den_ps = dpsum.tile([128, CHUNK], F32, tag="db", name="den_ps")
                self.p_pairs = {}

        def s_pair(st, u):
            # two S tiles into the two banks of one wide PSUM slot; ONE
            # 1024-wide exp activation -> fp8 P pair
            p_pair = ppool.tile([128, 2, CHUNK], F8, tag="p", name="p")
            st.p_pairs[u] = p_pair
            sp = spsum.tile([128, 2, CHUNK], F32, tag="s", name="sp")
            for s2 in range(2):
                t = 2 * u + s2
                ts = slice(t * 128, (t + 1) * 128)
                nc.tensor.matmul(sp[:, s2, :], x2_sb[:, 0, ts], q_sb[:, 0, st.cs], start=True, stop=False)
                nc.tensor.matmul(sp[:, s2, :], x2_sb[:, 1, ts], q_sb[:, 1, st.cs], start=False, stop=True)
            nc.scalar.activation(p_pair[:], sp[:], mybir.ActivationFunctionType.Exp, scale=SCALE)

        def emit_pv(st, u):
            # fp8 DoubleRow: one matmul covers both key tiles of the pair
            first = u == 0
            p = st.p_pairs.pop(u)
            # stop stays False: the bias-fold matmuls close the acc groups
            nc.tensor.matmul(st.acc0[:], v_sb[:, u, :, 0:128], p[:],
                             start=first, stop=False, perf_mode=DR)
            nc.tensor.matmul(st.acc1[:], v_sb[:, u, :, 128:256], p[:],
                             start=first, stop=False, perf_mode=DR)
            nc.tensor.matmul(st.den_ps[0:16, :], ones_pair[:], p[:],
                             start=first, stop=(u == NPAIRS - 1), perf_mode=DR)

        def flush_chunk(st):
            # the trailing PV pairs (the final one waits on its exp); then
            # reciprocal straight from PSUM while an f32r-rounded copy (ACT,
            # parallel with the DVE reciprocal) feeds the bias-fold matmuls
            for u in range(NPAIRS - PVTRAIL, NPAIRS):
                emit_pv(st, u)
            den_sb = dpool.tile([1, CHUNK], MM, tag="den_sb", name="den_sb")
            nc.scalar.copy(den_sb[:], st.den_ps[0:1, :])
            st.den_sb = den_sb
            recip_f32 = dpool.tile([1, CHUNK], F32, tag="recip_f32", name="recip_f32")
            nc.vector.reciprocal_approx_fast(out=recip_f32[:], in_=st.den_ps[0:1, :])
            st.recip_f32 = recip_f32

        # --- softmax tails. tail_a: bias-fold matmuls (acc_ct += bv_ct (x)
        # den) close the PV accumulation groups. tail_b: Pool-engine
        # partition broadcast of 1/den + normalize + out DMA. For chunks
        # 0..2 these are woven into the next chunk's S stream; non-final
        # out DMAs ride the Pool ring so the final chunk's two half DMAs
        # find empty Sync/Act rings. ---
        def tail_a(st):
            nc.tensor.matmul(st.acc0[:], bv_row[:, 0, :], st.den_sb[:], start=False, stop=True)
            nc.tensor.matmul(st.acc1[:], bv_row[:, 1, :], st.den_sb[:], start=False, stop=True)

        def tail_b(st, final=False):
            bcast_sb = opool.tile([128, CHUNK], F32, tag="bcast_sb", name="bcast_sb")
            nc.gpsimd.partition_broadcast(bcast_sb[:], st.recip_f32[:])
            if final:
                # bias already folded; separate tiles per half, DMAs split
                # across the (empty) Sync and Activation rings
                o0 = opool.tile([128, CHUNK], F32, tag="o_f0", name="o_f0")
                nc.vector.tensor_mul(o0[:], st.acc0[:], bcast_sb[:])
                nc.sync.dma_start(
                    out=split_h(out[:, st.cs])[:, 0:1, :],
                    in_=o0[:].rearrange("p (o w) -> p o w", o=1),
                )
                o1 = opool.tile([128, CHUNK], F32, tag="o_f1", name="o_f1")
                nc.vector.tensor_mul(o1[:], st.acc1[:], bcast_sb[:])
                nc.scalar.dma_start(
                    out=split_h(out[:, st.cs])[:, 1:2, :],
                    in_=o1[:].rearrange("p (o w) -> p o w", o=1),
                )
            else:
                o2 = opool.tile([128, 2, CHUNK], F32, tag="o2", name="o2")
                for ct, acc in ((0, st.acc0), (1, st.acc1)):
                    nc.vector.tensor_mul(o2[:, ct, :], acc[:], bcast_sb[:])
                nc.gpsimd.dma_start(out=split_h(out[:, st.cs]), in_=o2[:])

        # ================= program =================
        # Prologue: q chunk-0 projection, all V pairs, remaining q
        # projections — streamed behind the input DMA.
        qproj(0)
        for u in range(NPAIRS):
            vproj_pair(u)
        for c0 in range(1, NQ_CHUNKS):
            qproj(c0)

        # Main loop: chunks 0..3 in S pairs; the previous chunk's trailing
        # PV pairs and its tail are woven into pairs 1..3.
        prev = None
        for c0 in range(NQ_CHUNKS):
            st = ChunkState(c0)
            for u in range(NPAIRS):
                s_pair(st, u)
                if u == 1 and prev is not None:
                    flush_chunk(prev)
                if u == 2 and prev is not None:
                    tail_a(prev)
                if u == 3 and prev is not None:
                    tail_b(prev)
                    prev = None
                if u >= PVTRAIL:
                    emit_pv(st, u - PVTRAIL)
            prev = st

        # final chunk's tail is exposed: shortest possible chain
        flush_chunk(prev)
        tail_a(prev)
        tail_b(prev, final=True)

    nc.compile()
    return nc


def core_inputs(inputs, core):
    """Slice full-problem inputs for one core (numpy). Host-side weight
    fusion: A = Wq^T Wk and cq = Wk^T bq fold the K projection away."""
    b, h = core // 2, core % 2
    x1r = np.asarray(inputs["x1"], dtype=np.float32).reshape(B, C, N)
    x2r = np.asarray(inputs["x2"], dtype=np.float32).reshape(B, C, N)
    Wq = np.asarray(inputs["Wq"], dtype=np.float32)
    Wk = np.asarray(inputs["Wk"], dtype=np.float32)
    A = np.ascontiguousarray((Wq.T @ Wk).astype(np.float32))  # [ci, r]
    cqv = (Wk.T @ np.asarray(inputs["bq"], dtype=np.float32)).astype(np.float32)
    return {
        "x1c": np.ascontiguousarray(x1r[b][:, h * NQ : (h + 1) * NQ]),
        "x2c": np.ascontiguousarray(x2r[b]),
        "aT": A,
        "wvT": np.ascontiguousarray(np.asarray(inputs["Wv"], dtype=np.float32).T),
        "cq": cqv.reshape(C, 1).copy(),
        "bv": np.asarray(inputs["bv"], dtype=np.float32).reshape(C, 1).copy(),
    }


_NC_CACHE = {}


def get_nc():
    if "nc" not in _NC_CACHE:
        _NC_CACHE["nc"] = build_nc()
    return _NC_CACHE["nc"]


def kernel(**inputs) -> np.ndarray:
    """Full-problem entry point: full inputs in, full [4,256,64,64] f32 out."""
    nc = get_nc()
    in_maps = [core_inputs(inputs, core) for core in range(8)]
    res = run_bass_kernel_spmd(nc, in_maps, list(range(8)))
    full = np.zeros((B, C, N), np.float32)
    for core in range(8):
        b, h = core // 2, core % 2
        full[b][:, h * NQ : (h + 1) * NQ] = res.results[core]["out"]
    return full.reshape(B, C, H, W)
